# revision 44
# baseline (speedup 1.0000x reference)
"""DySimGCF message-passing kernel for 8 Trainium2 NeuronCores.

out[t, :] = sum_{e: to_e = t} norm_e * x[from_e, :]
norm_e = exp(a_e) / sqrt(Sin[to_e] * Sout[from_e])
Sin[t] = sum_{e: to_e = t} exp(a_e);  Sout[f] = sum_{e: from_e = f} exp(a_e)
(equivalent to the reference's max-stabilized segment softmaxes in exact
arithmetic; attrs are standard-normal so exp() cannot overflow in fp16)

Distribution (edge parallelism, target-sharded):
- Phase B: each core computes Sout for its 12.5K nodes via a dense
  max-degree-padded attr layout (no gather), builds xp[f] = fp16
  [x[f]/sqrt(Sout[f]) | 1 | 0pad] 128-col rows in NODE order, and
  AllGathers xp in two halves so the second collective overlaps Phase C.
- Phase C: edges sharded by TO-slice, bucketed into (128-target-block x
  source-chunk) cells.  Static structure per 8-block super-tile x chunk:
  3 full 128-edge groups per cell plus a small shared tail region for
  cell overflow (tail slots are packed, padded with negative gather
  indices at the end, and trimmed per-core via num_idxs_reg).  One fp16
  matmul per group accumulates Q.T @ [X | 1] into one PSUM bank per
  block, held open across all 4 chunks (2 passes); the final per-block
  scale by 1/sqrt(Sin) writes the output slice densely.
"""

import numpy as np

import concourse.bacc as bacc
import concourse.bass as bass
import concourse.mybir as mybir
import concourse.tile as tile
from concourse.bass_utils import run_bass_kernel_spmd


def _install_ntff_shim():
    """Register the NTFF profile hook so trace=True works even when the
    image's antenv lacks axon_hooks (harmless no-op when unavailable)."""
    try:
        import sys, types
        import antenv
        if "antenv.axon_hooks" not in sys.modules:
            mod = types.ModuleType("antenv.axon_hooks")
            mod._hook = None
            mod.set_axon_ntff_profile_hook = lambda h, _m=mod: setattr(
                _m, "_hook", h)
            mod.get_axon_ntff_profile_hook = lambda _m=mod: _m._hook
            sys.modules["antenv.axon_hooks"] = mod
            antenv.axon_hooks = mod
            from trn_agent_boot.trn_boot import _ntff_profile_via_ctypes
            h = _ntff_profile_via_ctypes("/opt/axon/libaxon_pjrt.so")
            if h is not None:
                mod.set_axon_ntff_profile_hook(h)
    except Exception:
        pass


_install_ntff_shim()

# Problem constants (nn_DySimGCF_18202071400771)
N = 100000
D = 64
DP = 128      # fp16 xp row width (256B: [x' 64 | one | 0pad 63])
DR = D + 1    # used rhs cols

C = 8         # cores
SL = N // C   # nodes per core = 12500
NB = -(-SL // 128)      # target blocks per core = 98
RPCN = NB * 128         # padded xp rows per core = 12544
HB = RPCN // 2          # half of a core's xp rows = 6272
CHROWS = 4 * HB         # rows per source chunk = 25088 (< 32768, int16-safe)
D3 = 3                  # static full groups per cell
SZS = [8] * (NB // 8) + ([NB % 8] if NB % 8 else [])  # super-tile sizes
NSTC = len(SZS)
EPS = 1e-20
EPS_B = 1e-6  # Phase-B Sout bias: keeps 1/sqrt finite in fp16 for deg-0 nodes
PAD_ATTR = -1000.0
PAD_TL = -512.0

XBUFS = 3       # X gather-tile pool depth (first XBUFS tiles are zeroed)
TRACE = False   # test.py may set kernel.TRACE = True
LAST_RESULT = None  # BassKernelResults of the last run (for test.py)

_PROGRAM_CACHE = {}


def _wrap16(idx):
    """[n] ints -> [128, n/16] int16 in the wrapped+replicated idx layout."""
    n = idx.shape[0]
    a = idx.reshape(n // 16, 16).T.astype(np.int16)
    return np.tile(a, (8, 1))


def _layout(edge_index, edge_attrs):
    """Host-side sharding/layout. Returns (meta, per-core input dict)."""
    f = edge_index[0].astype(np.int64)
    t = edge_index[1].astype(np.int64)
    a = edge_attrs.astype(np.float32)
    E = f.shape[0]

    # ---------------- Phase B structure (dense, node order) ----------------
    deg = np.bincount(f, minlength=N)
    DMAXB = int(deg.max())
    oc = f // SL
    eo = np.argsort(f, kind="stable")
    ef = f[eo]
    node_start = np.zeros(N + 1, np.int64)
    node_start[1:] = np.cumsum(np.bincount(ef, minlength=N))
    j_in_node = np.arange(E) - node_start[ef]
    floc = ef - (ef // SL) * SL
    attrB = np.full((C, 128, NB * DMAXB), PAD_ATTR, np.float32)
    attrB[ef // SL, floc % 128, (floc // 128) * DMAXB + j_in_node] = a[eo]

    # ---------------- Phase C structure ----------------
    c_src = f // SL
    l_src = f - c_src * SL
    hh = l_src // HB
    k = 2 * hh + (c_src >= 4)                 # source chunk 0..3
    lidx = (c_src % 4) * HB + (l_src - hh * HB)  # chunk-local xp row

    tcore = t // SL
    tloc = t - tcore * SL
    blk = tloc // 128
    trel = tloc % 128
    s_of = blk // 8                            # super-tile 0..NSTC-1
    brel = blk - 8 * s_of
    P = k // 2
    qi = k % 2
    ti = P * (2 * NSTC) + s_of * 2 + qi        # tile id 0..(4*NSTC-1)
    NT = 4 * NSTC

    cell = (tcore * 4 + k) * NB + blk
    ceo = np.argsort(cell, kind="stable")
    ccel = cell[ceo]
    cstart = np.zeros(C * 4 * NB + 1, np.int64)
    cstart[1:] = np.cumsum(np.bincount(ccel, minlength=C * 4 * NB))
    r_in_cell = np.arange(E) - cstart[ccel]

    tco = tcore[ceo]
    tio = ti[ceo]
    bro = brel[ceo]
    so = s_of[ceo]
    FULLCAP = D3 * 128
    is_full = r_in_cell < FULLCAP

    # tail ranks: excess edges ranked within (tcore, ti) sorted by (brel, r)
    exm = ~is_full
    exc_key = (tco[exm] * NT + tio[exm]) * (NB * 1024) + bro[exm] * 1024 + (
        r_in_cell[exm] - FULLCAP)
    exo = np.argsort(exc_key, kind="stable")
    ztile = tco[exm][exo] * NT + tio[exm][exo]
    tstart = np.zeros(C * NT + 1, np.int64)
    tstart[1:] = np.cumsum(np.bincount(ztile, minlength=C * NT))
    trank_sorted = np.arange(exm.sum()) - tstart[ztile]
    trank = np.empty(exm.sum(), np.int64)
    trank[exo] = trank_sorted
    tail_cnt = np.bincount(ztile, minlength=C * NT).reshape(C, NT)

    # per-super-tile tail capacity (shared across cores/chunks: static program)
    TCs = []
    for s in range(NSTC):
        cols = [P0 * (2 * NSTC) + s * 2 + q for P0 in range(2) for q in range(2)]
        m = int(tail_cnt[:, cols].max())
        TCs.append(max(1, -(-m // 128)))
    TCs = tuple(TCs)

    NGs = tuple(SZS[s] * D3 + TCs[s] for s in range(NSTC))
    gcb = np.zeros(NT + 1, np.int64)   # group col base per tile
    for t_i in range(NT):
        s = (t_i % (2 * NSTC)) // 2
        gcb[t_i + 1] = gcb[t_i] + NGs[s]
    GTOT = int(gcb[NT])

    # slot assignment
    slot_col = np.empty(E, np.int64)   # group column (global)
    slot_p = np.empty(E, np.int64)     # partition
    tlv = np.empty(E, np.float32)      # tloc encoding
    fm = is_full
    slot_col[fm] = gcb[tio[fm]] + bro[fm] * D3 + r_in_cell[fm] // 128
    slot_p[fm] = r_in_cell[fm] % 128
    tlv[fm] = trel[ceo][fm]
    szD3 = np.array([SZS[s] * D3 for s in range(NSTC)], np.int64)
    slot_col[exm] = gcb[tio[exm]] + szD3[so[exm]] + trank // 128
    slot_p[exm] = trank % 128
    tlv[exm] = bro[exm] * 128 + trel[ceo][exm]

    attrC = np.full((C, 128, GTOT), PAD_ATTR, np.float32)
    tlocC = np.full((C, 128, GTOT), PAD_TL, np.float16)
    idxF = np.zeros((C, GTOT * 128), np.int64)  # full-region pad -> row 0
    # tail region pads -> -1 (trailing, trimmed by num_idxs_reg/kernel)
    for t_i in range(NT):
        s = (t_i % (2 * NSTC)) // 2
        b0 = gcb[t_i] + SZS[s] * D3
        idxF[:, b0 * 128 : gcb[t_i + 1] * 128] = -1
    attrC[tco, slot_p, slot_col] = a[ceo]
    tlocC[tco, slot_p, slot_col] = tlv.astype(np.float16)
    idxF[tco, slot_col * 128 + slot_p] = lidx[ceo]

    gidxC = np.zeros((C, 128, GTOT * 8), np.int16)
    for cc in range(C):
        for t_i in range(NT):
            sl_ = slice(int(gcb[t_i]) * 128, int(gcb[t_i + 1]) * 128)
            gidxC[cc, :, gcb[t_i] * 8 : gcb[t_i + 1] * 8] = _wrap16(idxF[cc, sl_])

    cnts = np.zeros((C, NT), np.int32)
    for t_i in range(NT):
        s = (t_i % (2 * NSTC)) // 2
        cnts[:, t_i] = SZS[s] * D3 * 128 + tail_cnt[:, t_i]

    iota = np.tile(np.arange(1024, dtype=np.float16), (128, 1))

    meta = dict(DMAXB=DMAXB, TCs=TCs, GTOT=GTOT,
                gcb=tuple(int(x) for x in gcb))
    in_maps = []
    for cc in range(C):
        in_maps.append({
            "x_slice": None,  # filled by caller
            "attrB": attrB[cc],
            "attrC": attrC[cc],
            "tlocC": tlocC[cc],
            "gidxC": gidxC[cc],
            "cnts": cnts[cc : cc + 1],
            "iota": iota,
        })
    return meta, in_maps


def _build_program(meta):
    DMAXB = meta["DMAXB"]
    TCs = meta["TCs"]
    GTOT = meta["GTOT"]
    gcb = meta["gcb"]
    NT = 4 * NSTC
    NGMAX = max(SZS[s] * D3 + TCs[s] for s in range(NSTC))
    TCMAX = max(TCs)

    f16 = mybir.dt.float16
    f32 = mybir.dt.float32

    nc = bacc.Bacc("TRN2", target_bir_lowering=False, debug=False,
                   num_devices=C, num_swdge_queues=4)

    x_slice = nc.dram_tensor("x_slice", [RPCN, D], f32, kind="ExternalInput")
    attrB = nc.dram_tensor("attrB", [128, NB * DMAXB], f32, kind="ExternalInput")
    attrC = nc.dram_tensor("attrC", [128, GTOT], f32, kind="ExternalInput")
    tlocC = nc.dram_tensor("tlocC", [128, GTOT], f16, kind="ExternalInput")
    gidxC = nc.dram_tensor("gidxC", [128, GTOT * 8], mybir.dt.int16,
                           kind="ExternalInput")
    cnts = nc.dram_tensor("cnts", [1, NT], mybir.dt.int32, kind="ExternalInput")
    iota_d = nc.dram_tensor("iota", [128, 1024], f16, kind="ExternalInput")
    out = nc.dram_tensor("out", [SL, D], f32, kind="ExternalOutput")

    xpc = nc.dram_tensor("xpc", [RPCN, DP], f16)
    xp_half = [
        nc.dram_tensor(f"xp_{h}", [C * HB, DP], f16, addr_space="Shared")
        for h in range(2)
    ]

    nreg = [nc.alloc_register(mybir.EngineType.Pool, f"cnt{i}") for i in range(4)]

    with tile.TileContext(nc) as tc:
        with tc.tile_pool(name="cst", bufs=1) as cst:
            iota_t = cst.tile([128, 1024], f16)
            nc.sync.dma_start(iota_t[:], iota_d.ap())
            eps_t = cst.tile([128, 1], f32)
            nc.vector.memset(eps_t[:], EPS)
            epsb_t = cst.tile([128, 1], f32)
            nc.vector.memset(epsb_t[:], EPS_B)
            cnts_t = cst.tile([1, NT], mybir.dt.int32)
            nc.sync.dma_start(cnts_t[:], cnts.ap())

            # ---------------- Phase B (pipelined per half) ----------------
            with tc.tile_pool(name="bph", bufs=1) as bph:
                attrB_t = bph.tile([128, NB, DMAXB], f32)
                expB = bph.tile([128, NB, DMAXB], f32)
                sout = bph.tile([128, NB], f32)
                stdB = bph.tile([128, NB], f32)
                rB = bph.tile([128, NB], f32)
                xd = bph.tile([128, NB, D], f32)
                xps = bph.tile([128, NB, DP], f16)
                for h in range(2):
                    g0, g1 = h * (NB // 2), (h + 1) * (NB // 2)
                    ng = g1 - g0
                    bs = slice(g0, g1)
                    nc.sync.dma_start(
                        attrB_t[:, bs, :],
                        attrB.ap()[:, g0 * DMAXB : g1 * DMAXB]
                        .rearrange("p (b j) -> p b j", j=DMAXB))
                    nc.scalar.activation(expB[:, bs, :], attrB_t[:, bs, :],
                                         mybir.ActivationFunctionType.Exp)
                    nc.vector.tensor_reduce(sout[:, bs], expB[:, bs, :],
                                            axis=mybir.AxisListType.X,
                                            op=mybir.AluOpType.add)
                    nc.scalar.activation(stdB[:, bs], sout[:, bs],
                                         mybir.ActivationFunctionType.Sqrt,
                                         bias=epsb_t[:])
                    nc.vector.reciprocal(rB[:, bs], stdB[:, bs])
                    nc.sync.dma_start(
                        xd[:, bs, :],
                        x_slice.ap()[g0 * 128 : g1 * 128].rearrange(
                            "(g p) c -> p g c", p=128))
                    nc.vector.memset(xps[:, bs, D + 1 :], 0.0)
                    nc.vector.memset(xps[:, bs, D : D + 1], 1.0)
                    nc.vector.tensor_tensor(
                        xps[:, bs, 0:D], xd[:, bs, :],
                        rB[:, bs].unsqueeze(-1).broadcast_to([128, ng, D]),
                        mybir.AluOpType.mult)
                    nc.sync.dma_start(
                        xpc.ap()[g0 * 128 : g1 * 128].rearrange(
                            "(g p) c -> p g c", p=128),
                        xps[:, bs, :])
                    nc.gpsimd.collective_compute(
                        "AllGather", mybir.AluOpType.bypass,
                        replica_groups=[list(range(C))],
                        ins=[xpc.ap()[g0 * 128 : g1 * 128]],
                        outs=[xp_half[h].ap()])

            # ---------------- Phase C ----------------
            with (
                tc.tile_pool(name="xg", bufs=XBUFS) as xgp,
                tc.tile_pool(name="qg", bufs=3) as qgp,
                tc.tile_pool(name="meta_p", bufs=4) as mp,
                tc.tile_pool(name="accp", bufs=1) as accp,
                tc.tile_pool(name="psp", bufs=1, space="PSUM") as psp,
            ):
                accA = accp.tile([128, NB, DR], f32)
                stdA = accp.tile([128, NB], f32)
                rcA = accp.tile([128, NB], f32)
                oA = accp.tile([128, NB, D], f32)

                def normalize(b0, b1):
                    # out[rows of blocks b0:b1] = accA[..,0:D]/sqrt(Sin+eps)
                    nc.scalar.activation(stdA[:, b0:b1], accA[:, b0:b1, D],
                                         mybir.ActivationFunctionType.Sqrt,
                                         bias=eps_t[:])
                    nc.vector.reciprocal(rcA[:, b0:b1], stdA[:, b0:b1])
                    nc.vector.tensor_tensor(
                        oA[:, b0:b1, :], accA[:, b0:b1, 0:D],
                        rcA[:, b0:b1].unsqueeze(-1).broadcast_to(
                            [128, b1 - b0, D]),
                        mybir.AluOpType.mult)
                    gfull = min(b1, SL // 128)
                    if gfull > b0:
                        nc.sync.dma_start(
                            out.ap()[b0 * 128 : gfull * 128].rearrange(
                                "(g p) c -> p g c", p=128),
                            oA[:, b0:gfull, :])
                    if b1 * 128 > SL // 128 * 128:
                        nc.sync.dma_start(
                            out.ap()[SL // 128 * 128 : SL],
                            oA[0 : SL - SL // 128 * 128, SL // 128, :])

                x_tiles_seen = 0
                for P in range(2):
                    for s in range(NSTC):
                        sz = SZS[s]
                        TC = TCs[s]
                        NG = sz * D3 + TC
                        ps = [psp.tile([128, DR], f32, tag=f"ps{b}",
                                       name=f"ps{b}")
                              for b in range(sz)]
                        for qi in range(2):
                            t_i = P * (2 * NSTC) + s * 2 + qi
                            gb = gcb[t_i]
                            nf = sz * D3
                            gi = mp.tile([128, NGMAX * 8], mybir.dt.int16,
                                         tag="gic")
                            nc.sync.dma_start(
                                gi[:, : NG * 8],
                                gidxC.ap()[:, gb * 8 : (gb + NG) * 8])
                            at = mp.tile([128, NGMAX], f32, tag="atc")
                            nc.sync.dma_start(at[:, :NG],
                                              attrC.ap()[:, gb : gb + NG])
                            tl = mp.tile([128, NGMAX], f16, tag="tlc")
                            nc.sync.dma_start(tl[:, :NG],
                                              tlocC.ap()[:, gb : gb + NG])
                            ex = mp.tile([128, NGMAX], f16, tag="exc")
                            nc.scalar.activation(
                                ex[:, :NG], at[:, :NG],
                                mybir.ActivationFunctionType.Exp)
                            rg = nreg[t_i % 4]
                            nc.gpsimd.reg_load(rg, cnts_t[0:1, t_i : t_i + 1])
                            X = xgp.tile([128, NGMAX, DP], f16, tag="X")
                            # trimmed tail slots are never gathered; SBUF
                            # garbage there could be NaN/inf and 0*NaN = NaN
                            # in the matmul.  The pool cycles 3 buffers, so a
                            # one-time memset of each (hidden under Phase B)
                            # guarantees finite stale data forever after.
                            if x_tiles_seen < XBUFS:
                                nc.vector.memset(X[:], 0.0)
                                x_tiles_seen += 1
                            nc.gpsimd.dma_gather(
                                out_ap=X[:, :NG, :],
                                in_ap=xp_half[P].ap()[qi * CHROWS :
                                                      (qi + 1) * CHROWS],
                                idxs_ap=gi[:, : NG * 8],
                                num_idxs=NG * 128, num_idxs_reg=rg,
                                elem_size=DP, single_packet=False,
                                queue_num=t_i % 4)
                            # Xs = exp(a_e) * [x' | 1] rows (includes Sin col).
                            # Emitted first so X frees earliest (gather WAR).
                            Xs = qgp.tile([128, NGMAX, DR], f16, tag="Xs")
                            nc.vector.tensor_tensor(
                                Xs[:, :NG, :], X[:, :NG, 0:DR],
                                ex[:, :NG].unsqueeze(-1).broadcast_to(
                                    [128, NG, DR]),
                                mybir.AluOpType.mult)
                            # one-hot builds (Q stays 0/1; exp folded into Xs)
                            Qf = qgp.tile([128, SZS[0] * D3, 128], f16, tag="Qf")
                            nc.vector.tensor_tensor(
                                Qf[:, :nf, :],
                                iota_t[:, 0:128].unsqueeze(1).broadcast_to(
                                    [128, nf, 128]),
                                tl[:, :nf].unsqueeze(-1).broadcast_to(
                                    [128, nf, 128]),
                                mybir.AluOpType.is_equal)
                            Qt = qgp.tile([128, TCMAX, SZS[0] * 128], f16,
                                          tag="Qt")
                            nc.vector.tensor_tensor(
                                Qt[:, :TC, : sz * 128],
                                iota_t[:, 0 : sz * 128].unsqueeze(1).broadcast_to(
                                    [128, TC, sz * 128]),
                                tl[:, nf : nf + TC].unsqueeze(-1).broadcast_to(
                                    [128, TC, sz * 128]),
                                mybir.AluOpType.is_equal)
                            if qi == 0:
                                for b in range(sz):
                                    for dd in range(D3):
                                        nc.tensor.matmul(
                                            out=ps[b][:],
                                            lhsT=Qf[:, b * D3 + dd, :],
                                            rhs=Xs[:, b * D3 + dd, :],
                                            start=(dd == 0), stop=False)
                                for tcg in range(TC):
                                    for b in range(sz):
                                        nc.tensor.matmul(
                                            out=ps[b][:],
                                            lhsT=Qt[:, tcg,
                                                    b * 128 : (b + 1) * 128],
                                            rhs=Xs[:, nf + tcg, :],
                                            start=False, stop=False)
                            else:
                                # last chunk: bank-major order so each bank's
                                # drain overlaps the other banks' matmuls
                                for b in range(sz):
                                    for dd in range(D3):
                                        nc.tensor.matmul(
                                            out=ps[b][:],
                                            lhsT=Qf[:, b * D3 + dd, :],
                                            rhs=Xs[:, b * D3 + dd, :],
                                            start=False, stop=False)
                                    for tcg in range(TC):
                                        nc.tensor.matmul(
                                            out=ps[b][:],
                                            lhsT=Qt[:, tcg,
                                                    b * 128 : (b + 1) * 128],
                                            rhs=Xs[:, nf + tcg, :],
                                            start=False, stop=(tcg == TC - 1))
                                    # immediate drain of this bank
                                    blkid = s * 8 + b
                                    if P == 0:
                                        # ACT engine: keeps the PSUM drain
                                        # out of the in-order DVE queue
                                        nc.scalar.activation(
                                            accA[:, blkid, :], ps[b][:],
                                            mybir.ActivationFunctionType.Copy)
                                    else:
                                        nc.vector.tensor_add(
                                            accA[:, blkid, :],
                                            accA[:, blkid, :], ps[b][:])
                        if P == 1 and s == 3:
                            normalize(0, 32)
                        if P == 1 and s == 8:
                            normalize(32, 72)

                normalize(72, NB)

    nc.compile()
    return nc


def kernel(x, edge_index, edge_attrs):
    global LAST_RESULT
    meta, in_maps = _layout(edge_index, edge_attrs)
    key = tuple(sorted((k, v) for k, v in meta.items() if k != "gcb")) + (
        meta["gcb"],)
    if key not in _PROGRAM_CACHE:
        _PROGRAM_CACHE[key] = _build_program(meta)
    nc = _PROGRAM_CACHE[key]
    xf = np.zeros((C, RPCN, D), np.float32)
    xs = np.ascontiguousarray(x, dtype=np.float32).reshape(C, SL, D)
    xf[:, :SL, :] = xs
    for cc in range(C):
        in_maps[cc]["x_slice"] = xf[cc]
    res = run_bass_kernel_spmd(nc, in_maps, core_ids=list(range(C)), trace=TRACE)
    LAST_RESULT = res
    return np.concatenate([res.results[cc]["out"] for cc in range(C)], axis=0)
